# revision 19
# baseline (speedup 1.0000x reference)
"""Trainium2 Bass kernel for nn_KernelAttnCoef (linear attention variant).

Math (per batch b, head h):
    A_h = q_h @ k_h^T                      # [n, n]   (n=256, d=16)
    qk_sum_h[k] = sum_l A_h[k, l]          # normalizer (== q_h . sum_l k_h)
    att_h = (A_h / qk_sum_h[:, None]) @ v_h    # v_h: [n(l), n(t), 8]
    out[b, k, t, 8h+j] = att_h[k, t, j]

Sharding: batch-parallel, core b <- batch b. The tiny normalized
A^T (2MB/core) is computed on the host; the device streams the 16MB v
through the tensor engine against per-head A^T weights and writes the
16MB output, both as large contiguous DMAs (memory-bound regime).
"""

import numpy as np

B = 8
N = 256
H = 8
DQK = 16
DVH = 8
DV = 64
EPS = 1e-05

MODE = "f32"  # "f32" | "f32r" | "bf16x3"
TC = 64       # t-tile size (TC*DVH = 512 = one PSUM bank of fp32)
# t-chunk plan: small edge chunks ramp the DMA/PE pipeline up and down
# quickly (startup/tail latency), big middle chunks keep DMA efficiency.
CHUNKS = [32, 64, 64, 64, 32]
V_BUFS = 3

_cache = {}


def _build(mode):
    from contextlib import ExitStack

    import concourse.tile as tile
    from concourse import bacc, mybir

    nc = bacc.Bacc("TRN2", target_bir_lowering=False, debug=False, num_devices=8)
    if mode in ("bf16x3", "bf16p", "bf16b"):
        dt_in, n_planes = mybir.dt.bfloat16, 2
        terms = [(0, 0), (0, 1), (1, 0)]  # (at_plane, v_plane): hh + hl + lh
    elif mode == "f32r":
        dt_in, n_planes = mybir.dt.float32r, 1
        terms = [(0, 0)]
    else:
        dt_in, n_planes = mybir.dt.float32, 1
        terms = [(0, 0)]
    packed = mode == "bf16p"  # v planes element-interleaved: [l, (t c plane)]
    blockp = mode == "bf16b"  # planes block-concatenated per (lc, tc) chunk

    at_shape = [2, 128, 2 * H * N] if blockp else [n_planes, 2, 128, H * N]
    at_d = nc.dram_tensor("at", at_shape, dt_in, kind="ExternalInput").ap()
    vw = 2 if (packed or blockp) else 1
    if blockp:
        v_shape = [2, 128, N * DV * 2]
    elif packed:
        v_shape = [1, N, N * DV * 2]
    else:
        v_shape = [n_planes, N, N * DV]
    v_d = nc.dram_tensor("v", v_shape, dt_in, kind="ExternalInput").ap()
    out_d = nc.dram_tensor(
        "out", [N, N * DV], mybir.dt.float32, kind="ExternalOutput"
    ).ap()

    chunks = CHUNKS if blockp else [TC] * (N // TC)
    assert sum(chunks) == N and all(c <= 64 for c in chunks)
    starts = [sum(chunks[:i]) for i in range(len(chunks))]
    FW = TC * DV  # max free width of one (lc) v tile / out tile

    with tile.TileContext(nc) as tc:
        with ExitStack() as ctx:
            at_pool = ctx.enter_context(tc.tile_pool(name="at", bufs=1))
            v_pool = ctx.enter_context(tc.tile_pool(name="v", bufs=V_BUFS))
            o_pool = ctx.enter_context(tc.tile_pool(name="o", bufs=2))
            ps_pool = ctx.enter_context(
                tc.tile_pool(name="ps", bufs=8, space="PSUM")
            )

            at_sb = {}
            if blockp:
                for lc in range(2):
                    t = at_pool.tile([128, 2 * H * N], dt_in, tag=f"at{lc}")
                    nc.sync.dma_start(out=t[:], in_=at_d[lc])
                    at_sb[0, lc] = t
                    at_sb[1, lc] = t
            else:
                for p in range(n_planes):
                    for lc in range(2):
                        t = at_pool.tile([128, H * N], dt_in, tag=f"at{p}{lc}")
                        nc.sync.dma_start(out=t[:], in_=at_d[p, lc])
                        at_sb[p, lc] = t

            nmm = 2 * len(terms)
            for tci, (ts_, tl) in enumerate(zip(starts, chunks)):
                fw = tl * DV
                vt = {}
                if blockp:
                    for lc in range(2):
                        t = v_pool.tile([128, 2 * fw], dt_in, tag=f"v{lc}")
                        nc.sync.dma_start(
                            out=t[:],
                            in_=v_d[lc, :, ts_ * 2 * DV : (ts_ + tl) * 2 * DV],
                        )
                        vt[0, lc] = t
                        vt[1, lc] = t
                else:
                    for p in range(1 if packed else n_planes):
                        for lc in range(2):
                            t = v_pool.tile([128, fw * vw], dt_in, tag=f"v{p}{lc}")
                            nc.sync.dma_start(
                                out=t[:],
                                in_=v_d[
                                    p,
                                    lc * 128 : (lc + 1) * 128,
                                    ts_ * DV * vw : (ts_ + tl) * DV * vw,
                                ],
                            )
                            vt[p, lc] = t
                            if packed:
                                vt[1, lc] = t
                for kc in range(2):
                    ot = o_pool.tile([128, fw], mybir.dt.float32, tag=f"o{kc}")
                    ov = ot[:].rearrange("p (t c) -> p t c", c=DV)
                    for h in range(H):
                        ps = ps_pool.tile([128, tl * DVH], mybir.dt.float32, tag="ps")
                        i = 0
                        for lc in range(2):
                            for (ap_, vp) in terms:
                                if packed:
                                    rhs = vt[vp, lc][:].rearrange(
                                        "p (t c s) -> p t c s", c=DV, s=2
                                    )[:, :, h * DVH : (h + 1) * DVH, vp]
                                elif blockp:
                                    rhs = vt[vp, lc][:, vp * fw : (vp + 1) * fw].rearrange(
                                        "p (t c) -> p t c", c=DV
                                    )[:, :, h * DVH : (h + 1) * DVH]
                                else:
                                    rhs = vt[vp, lc][:].rearrange(
                                        "p (t c) -> p t c", c=DV
                                    )[:, :, h * DVH : (h + 1) * DVH]
                                lhs_off = (ap_ * H * N if blockp else 0) + h * N + kc * 128
                                nc.tensor.matmul(
                                    ps[:],
                                    lhsT=at_sb[ap_, lc][:, lhs_off : lhs_off + 128],
                                    rhs=rhs,
                                    start=(i == 0),
                                    stop=(i == nmm - 1),
                                )
                                i += 1
                        dst = ov[:, :, h * DVH : (h + 1) * DVH]
                        src = ps[:].rearrange("p (t j) -> p t j", j=DVH)
                        if h % 2 == 0:
                            nc.vector.tensor_copy(out=dst, in_=src)
                        else:
                            nc.scalar.copy(out=dst, in_=src)
                    nc.sync.dma_start(
                        out=out_d[
                            kc * 128 : (kc + 1) * 128, ts_ * DV : (ts_ + tl) * DV
                        ],
                        in_=ot[:],
                    )
    nc.compile()
    return nc


def _get_nc(mode=None):
    mode = mode or MODE
    if mode not in _cache:
        _cache[mode] = _build(mode)
    return _cache[mode]


def _qk_sums(query, key):
    """Replicate the reference's fp32 normalizer computation bit-exactly
    (it is severely cancellation-amplified for near-zero sums, so matching
    the fp32 op order matters more than extra precision)."""
    import jax.numpy as jnp

    q32 = jnp.asarray(np.asarray(query, np.float32))
    k32 = jnp.asarray(np.asarray(key, np.float32))
    q_rs = jnp.stack(jnp.split(q32, H, axis=-1), axis=0)  # [H,B,n,d]
    k_rs = jnp.stack(jnp.split(k32, H, axis=-1), axis=0)
    k_sum = k_rs.sum(axis=2)  # [H,B,d]
    qk_sum = jnp.einsum('hbki,hbi->hbk', q_rs, k_sum)  # [H,B,n]
    qk_sum = jnp.where(qk_sum == 0, EPS, qk_sum)
    return np.asarray(qk_sum)  # [H, B, n]


def _prep_inputs(query, key, value, mode=None):
    """Host prep: per-core (per-batch) input maps."""
    mode = mode or MODE
    qk_all = _qk_sums(query, key)
    in_maps = []
    for b in range(B):
        qb = np.asarray(query[b], np.float64)
        kb = np.asarray(key[b], np.float64)
        at = np.empty((2, 128, H, N), np.float64)  # [lc, l, h, k]
        for h in range(H):
            qh = qb[:, h * DQK : (h + 1) * DQK]
            kh = kb[:, h * DQK : (h + 1) * DQK]
            A = qh @ kh.T  # [k, l]
            qk = qk_all[h, b].astype(np.float64)
            atp = (A / qk[:, None]).T  # [l, k]
            at[0, :, h, :] = atp[:128]
            at[1, :, h, :] = atp[128:]
        at = at.reshape(2, 128, H * N)
        vb = np.asarray(value[b], np.float32).reshape(N, N * DV)
        if mode in ("bf16x3", "bf16p", "bf16b"):
            import ml_dtypes

            bf16 = ml_dtypes.bfloat16
            a32 = at.astype(np.float32)
            ah = a32.astype(bf16)
            al = (a32 - ah.astype(np.float32)).astype(bf16)
            vh = vb.astype(bf16)
            vl = (vb - vh.astype(np.float32)).astype(bf16)
            if mode == "bf16b":
                ahl = np.concatenate([ah, al], axis=2)  # [2, 128, 2*H*N]
                vh2 = vh.reshape(2, 128, N * DV)
                vl2 = vl.reshape(2, 128, N * DV)
                blocks = []
                ts_ = 0
                for tl in CHUNKS:
                    blocks.append(vh2[:, :, ts_ * DV : (ts_ + tl) * DV])
                    blocks.append(vl2[:, :, ts_ * DV : (ts_ + tl) * DV])
                    ts_ += tl
                vpk = np.ascontiguousarray(np.concatenate(blocks, axis=2))
                in_maps.append({"at": ahl, "v": vpk})
            elif mode == "bf16p":
                vp = np.empty((N, N * DV, 2), bf16)
                vp[:, :, 0] = vh
                vp[:, :, 1] = vl
                in_maps.append(
                    {"at": np.stack([ah, al]), "v": vp.reshape(1, N, N * DV * 2)}
                )
            else:
                in_maps.append(
                    {"at": np.stack([ah, al]), "v": np.stack([vh, vl])}
                )
        else:
            in_maps.append(
                {"at": at.astype(np.float32)[None], "v": vb[None]}
            )
    return in_maps


def kernel(query, key, value):
    from concourse.bass_utils import run_bass_kernel_spmd

    nc = _get_nc()
    in_maps = _prep_inputs(query, key, value)
    res = run_bass_kernel_spmd(nc, in_maps, list(range(B)))
    return np.stack(
        [res.results[b]["out"].reshape(N, N, DV) for b in range(B)]
    )


# revision 23
# speedup vs baseline: 1.1668x; 1.1668x over previous
"""Trainium2 Bass kernel for nn_KernelAttnCoef (linear attention variant).

Math (per batch b, head h):
    A_h = q_h @ k_h^T                      # [n, n]   (n=256, d=16)
    qk_sum_h[k] = sum_l A_h[k, l]          # normalizer (== q_h . sum_l k_h)
    att_h = (A_h / qk_sum_h[:, None]) @ v_h    # v_h: [n(l), n(t), 8]
    out[b, k, t, 8h+j] = att_h[k, t, j]

Sharding: batch-parallel, core b <- batch b. The tiny normalized
A^T (2MB/core) is computed on the host; the device streams the 16MB v
through the tensor engine against per-head A^T weights and writes the
16MB output, both as large contiguous DMAs (memory-bound regime).
"""

import numpy as np

B = 8
N = 256
H = 8
DQK = 16
DVH = 8
DV = 64
EPS = 1e-05

MODE = "f32"  # "f32" | "f32r" | "bf16x3"
TC = 64       # t-tile size (TC*DVH = 512 = one PSUM bank of fp32)
# t-chunk plan: small edge chunks ramp the DMA/PE pipeline up and down
# quickly (startup/tail latency), big middle chunks keep DMA efficiency.
CHUNKS = [32, 64, 64, 64, 32]
V_BUFS = 3

_cache = {}


def _build(mode):
    from contextlib import ExitStack

    import concourse.tile as tile
    from concourse import bacc, mybir

    nc = bacc.Bacc("TRN2", target_bir_lowering=False, debug=False, num_devices=8)
    if mode in ("bf16x3", "bf16p", "bf16b"):
        dt_in, n_planes = mybir.dt.bfloat16, 2
        terms = [(0, 0), (0, 1), (1, 0)]  # (at_plane, v_plane): hh + hl + lh
    elif mode == "f32r":
        dt_in, n_planes = mybir.dt.float32r, 1
        terms = [(0, 0)]
    else:
        dt_in, n_planes = mybir.dt.float32, 1
        terms = [(0, 0)]
    packed = mode == "bf16p"  # v planes element-interleaved: [l, (t c plane)]
    blockp = mode == "bf16b"  # planes block-concatenated per (lc, tc) chunk

    at_shape = [2, 128, 2 * H * N] if blockp else [n_planes, 2, 128, H * N]
    at_d = nc.dram_tensor("at", at_shape, dt_in, kind="ExternalInput").ap()
    vw = 2 if (packed or blockp) else 1
    if blockp:
        v_shape = [2, 128, N * DV * 2]
    elif packed:
        v_shape = [1, N, N * DV * 2]
    else:
        v_shape = [n_planes, N, N * DV]
    v_d = nc.dram_tensor("v", v_shape, dt_in, kind="ExternalInput").ap()
    out_d = nc.dram_tensor(
        "out", [N, N * DV], mybir.dt.float32, kind="ExternalOutput"
    ).ap()

    chunks = CHUNKS if blockp else [TC] * (N // TC)
    assert sum(chunks) == N and all(c <= 64 for c in chunks)
    starts = [sum(chunks[:i]) for i in range(len(chunks))]
    FW = TC * DV  # max free width of one (lc) v tile / out tile

    with tile.TileContext(nc) as tc:
        with ExitStack() as ctx:
            at_pool = ctx.enter_context(tc.tile_pool(name="at", bufs=1))
            v_pool = ctx.enter_context(tc.tile_pool(name="v", bufs=V_BUFS))
            o_pool = ctx.enter_context(tc.tile_pool(name="o", bufs=2))
            ps_pool = ctx.enter_context(
                tc.tile_pool(name="ps", bufs=8, space="PSUM")
            )

            at_sb = {}
            if blockp:
                # one [128, 4*H*N] tile: [ah-lc0 | al-lc0 | ah-lc1 | al-lc1]
                t = at_pool.tile([128, 4 * H * N], dt_in, tag="at")
                for lc in range(2):
                    nc.sync.dma_start(
                        out=t[:, lc * 2 * H * N : (lc + 1) * 2 * H * N],
                        in_=at_d[lc],
                    )
                    at_sb[0, lc] = t
                    at_sb[1, lc] = t
            else:
                for p in range(n_planes):
                    for lc in range(2):
                        t = at_pool.tile([128, H * N], dt_in, tag=f"at{p}{lc}")
                        nc.sync.dma_start(out=t[:], in_=at_d[p, lc])
                        at_sb[p, lc] = t

            nmm = 2 * len(terms)
            for tci, (ts_, tl) in enumerate(zip(starts, chunks)):
                fw = tl * DV
                vt = {}
                if blockp:
                    for lc in range(2):
                        t = v_pool.tile([128, 2 * fw], dt_in, tag=f"v{lc}")
                        nc.sync.dma_start(
                            out=t[:],
                            in_=v_d[lc, :, ts_ * 2 * DV : (ts_ + tl) * 2 * DV],
                        )
                        vt[0, lc] = t
                        vt[1, lc] = t
                else:
                    for p in range(1 if packed else n_planes):
                        for lc in range(2):
                            t = v_pool.tile([128, fw * vw], dt_in, tag=f"v{p}{lc}")
                            nc.sync.dma_start(
                                out=t[:],
                                in_=v_d[
                                    p,
                                    lc * 128 : (lc + 1) * 128,
                                    ts_ * DV * vw : (ts_ + tl) * DV * vw,
                                ],
                            )
                            vt[p, lc] = t
                            if packed:
                                vt[1, lc] = t
                for kc in range(2):
                    ot = o_pool.tile([128, fw], mybir.dt.float32, tag=f"o{kc}")
                    ov = ot[:].rearrange("p (t c) -> p t c", c=DV)
                    for h in range(H):
                        ps = ps_pool.tile([128, tl * DVH], mybir.dt.float32, tag="ps")
                        i = 0
                        for lc in range(2):
                            for (ap_, vp) in terms:
                                if packed:
                                    rhs = vt[vp, lc][:].rearrange(
                                        "p (t c s) -> p t c s", c=DV, s=2
                                    )[:, :, h * DVH : (h + 1) * DVH, vp]
                                elif blockp:
                                    rhs = vt[vp, lc][:, vp * fw : (vp + 1) * fw].rearrange(
                                        "p (t c) -> p t c", c=DV
                                    )[:, :, h * DVH : (h + 1) * DVH]
                                else:
                                    rhs = vt[vp, lc][:].rearrange(
                                        "p (t c) -> p t c", c=DV
                                    )[:, :, h * DVH : (h + 1) * DVH]
                                lhs_off = (
                                    (lc * 2 + ap_) * H * N if blockp else 0
                                ) + h * N + kc * 128
                                nc.tensor.matmul(
                                    ps[:],
                                    lhsT=at_sb[ap_, lc][:, lhs_off : lhs_off + 128],
                                    rhs=rhs,
                                    start=(i == 0),
                                    stop=(i == nmm - 1),
                                )
                                i += 1
                        nc.vector.tensor_copy(
                            out=ov[:, :, h * DVH : (h + 1) * DVH],
                            in_=ps[:].rearrange("p (t j) -> p t j", j=DVH),
                        )
                    # stores issue from the ACT sequencer so a blocked v-load
                    # wait on the sync sequencer can't stall store issue
                    nc.scalar.dma_start(
                        out=out_d[
                            kc * 128 : (kc + 1) * 128, ts_ * DV : (ts_ + tl) * DV
                        ],
                        in_=ot[:],
                    )
    nc.compile()
    return nc


def _get_nc(mode=None):
    mode = mode or MODE
    if mode not in _cache:
        _cache[mode] = _build(mode)
    return _cache[mode]


def _qk_sums(query, key):
    """Replicate the reference's fp32 normalizer computation bit-exactly
    (it is severely cancellation-amplified for near-zero sums, so matching
    the fp32 op order matters more than extra precision)."""
    import jax.numpy as jnp

    q32 = jnp.asarray(np.asarray(query, np.float32))
    k32 = jnp.asarray(np.asarray(key, np.float32))
    q_rs = jnp.stack(jnp.split(q32, H, axis=-1), axis=0)  # [H,B,n,d]
    k_rs = jnp.stack(jnp.split(k32, H, axis=-1), axis=0)
    k_sum = k_rs.sum(axis=2)  # [H,B,d]
    qk_sum = jnp.einsum('hbki,hbi->hbk', q_rs, k_sum)  # [H,B,n]
    qk_sum = jnp.where(qk_sum == 0, EPS, qk_sum)
    return np.asarray(qk_sum)  # [H, B, n]


def _prep_inputs(query, key, value, mode=None):
    """Host prep: per-core (per-batch) input maps."""
    mode = mode or MODE
    qk_all = _qk_sums(query, key)
    in_maps = []
    for b in range(B):
        qb = np.asarray(query[b], np.float64)
        kb = np.asarray(key[b], np.float64)
        at = np.empty((2, 128, H, N), np.float64)  # [lc, l, h, k]
        for h in range(H):
            qh = qb[:, h * DQK : (h + 1) * DQK]
            kh = kb[:, h * DQK : (h + 1) * DQK]
            A = qh @ kh.T  # [k, l]
            qk = qk_all[h, b].astype(np.float64)
            atp = (A / qk[:, None]).T  # [l, k]
            at[0, :, h, :] = atp[:128]
            at[1, :, h, :] = atp[128:]
        at = at.reshape(2, 128, H * N)
        vb = np.asarray(value[b], np.float32).reshape(N, N * DV)
        if mode in ("bf16x3", "bf16p", "bf16b"):
            import ml_dtypes

            bf16 = ml_dtypes.bfloat16
            a32 = at.astype(np.float32)
            ah = a32.astype(bf16)
            al = (a32 - ah.astype(np.float32)).astype(bf16)
            vh = vb.astype(bf16)
            vl = (vb - vh.astype(np.float32)).astype(bf16)
            if mode == "bf16b":
                ahl = np.concatenate([ah, al], axis=2)  # [2, 128, 2*H*N]
                vh2 = vh.reshape(2, 128, N * DV)
                vl2 = vl.reshape(2, 128, N * DV)
                blocks = []
                ts_ = 0
                for tl in CHUNKS:
                    blocks.append(vh2[:, :, ts_ * DV : (ts_ + tl) * DV])
                    blocks.append(vl2[:, :, ts_ * DV : (ts_ + tl) * DV])
                    ts_ += tl
                vpk = np.ascontiguousarray(np.concatenate(blocks, axis=2))
                in_maps.append({"at": ahl, "v": vpk})
            elif mode == "bf16p":
                vp = np.empty((N, N * DV, 2), bf16)
                vp[:, :, 0] = vh
                vp[:, :, 1] = vl
                in_maps.append(
                    {"at": np.stack([ah, al]), "v": vp.reshape(1, N, N * DV * 2)}
                )
            else:
                in_maps.append(
                    {"at": np.stack([ah, al]), "v": np.stack([vh, vl])}
                )
        else:
            in_maps.append(
                {"at": at.astype(np.float32)[None], "v": vb[None]}
            )
    return in_maps


def kernel(query, key, value):
    from concourse.bass_utils import run_bass_kernel_spmd

    nc = _get_nc()
    in_maps = _prep_inputs(query, key, value)
    res = run_bass_kernel_spmd(nc, in_maps, list(range(B)))
    return np.stack(
        [res.results[b]["out"].reshape(N, N, DV) for b in range(B)]
    )
